# revision 16
# baseline (speedup 1.0000x reference)
"""Bathtub reconstructor Trainium2 kernel.

Reference does, per (b, y, x, t) cell with its 16 fine topo values z_k:
    solve mean(relu(h - z)) = d by 20-step bisection, output relu(h - z_k).

Key identity (water-filling): with z sorted ascending and P_j = z_1+...+z_j,
    sum_k relu(h - z_k) = max_j (j*h - P_j)   (convex, increasing)
so the root of sum = 16*d is exactly
    h* = min_{j=1..16} (16*d + P_j) / j = min_j (a_j * d + b_j),
with a_j = 16/j (constants) and b_j = P_j/j (per-cell constants).
This replaces the 20-iteration bisection with 16 fused multiply-adds and a
16-way min, then the output pass relu(h* - z_k).

Device layout (per core, n_y sharded 8 ways -> 8 y-rows/core):
  partitions = 128 cells (4 tiles cover the 512 (y,x) cells)
  free dim   = 512 combos (b-major: b*32 + t)
  stage1: hj[j] = a_j*d + b_j   (scalar ACT: Identity w/ scale+bias, and
                                 vector tensor_scalar mult+add, split)
  stage2: h = min over j        (vector tensor_reduce, j innermost via AP)
  stage3: out[k] = relu(h - z_k) (vector tensor_scalar add+max / ACT Relu)
All DMAs fully contiguous; host pre/post-permutes (cheap numpy).
"""

import numpy as np

import concourse.bass as bass
import concourse.tile as tile
from concourse import bacc, dve_ops, mybir
from concourse.bass_utils import run_bass_kernel_spmd
from concourse.dve_ops import OPS, DveOp, get_dve_sub_opcode, has_src1
from concourse.dve_spec import C0, C1, Spec, Src0, Src1, lower, minn
from concourse.dve_uop import DveOpSpec


def _register_affine_min() -> DveOp:
    """Custom fused DVE op: out = min(in0*s0 + s1, in1).

    One [128,512] 1x-rate instruction per water-level line replaces a
    tensor_scalar (affine) + tensor_tensor (min-tree level) pair: the
    16-line lower envelope becomes a 15-op min-accumulate chain.
    """
    for o in OPS:
        if o.name == "AFFINE_THEN_MIN":
            return o
    spec = Spec(
        body=minn(Src0 * C0 + C1, Src1),
        reference=lambda in0, in1, s0, s1, imm2: np.minimum(
            in0.astype(np.float32) * s0 + s1, in1
        ),
    )
    op = DveOp("AFFINE_THEN_MIN", spec, subdim=False, uops_sha={})
    OPS.append(op)
    dve_ops.CUSTOM_DVE_SPECS[op.name] = op.spec
    dve_ops._SUB_OPCODE_FOR_NAME[op.name] = (
        dve_ops._CUSTOM_DVE_ROW_BASE + len(OPS) - 1
    )
    for ver in ("v3", "v4"):
        tmp = DveOpSpec(
            name=op.name,
            opcode=get_dve_sub_opcode(op.name),
            uops=lower(spec, ver=ver),
            rd1_en=has_src1(spec),
        )
        op.uops_sha[ver] = tmp.sha(ver)
    return op

BS, NY, NX, NT, F = 16, 64, 64, 32, 4
FF = F * F                # 16 fine cells per coarse cell
NCORES = 8
YPC = NY // NCORES        # 8 coarse y rows per core
CELLS = YPC * NX          # 512 cells per core
NCT = CELLS // 128        # 4 cell-tiles of 128 partitions
COMBOS = BS * NT          # 512 (b, t) combos per cell

F32 = mybir.dt.float32

# Engine split: vector runs the fused affine+min chain (stage1+2), scalar
# runs stage3 relu acts (~707ns each). GpSimd is unusable here: its
# tensor_scalar path measured 8.3us/op and its SBUF-port contention
# starved the DVE 12x.
S3_VEC = 0    # stage3: last S3_VEC k's on vector, rest on scalar

_CACHE = {}


def _build_nc():
    fmin = _register_affine_min()
    nc = bacc.Bacc(
        "TRN2", target_bir_lowering=False, debug=False, num_devices=NCORES
    )
    u_ext = nc.declare_dram_parameter("u", [CELLS, COMBOS], F32, isOutput=False)
    cf_ext = nc.declare_dram_parameter("coef", [CELLS, FF], F32, isOutput=False)
    nz_ext = nc.declare_dram_parameter("negz", [CELLS, FF], F32, isOutput=False)
    out_ext = nc.declare_dram_parameter(
        "out", [CELLS, FF * COMBOS], F32, isOutput=True
    )

    a = [float(FF) / j for j in range(1, FF + 1)]

    with tile.TileContext(nc) as tc:
        with (
            tc.tile_pool(name="dpool", bufs=3) as dpool,
            tc.tile_pool(name="cfpool", bufs=3) as cfpool,
            tc.tile_pool(name="nzpool", bufs=3) as nzpool,
            tc.tile_pool(name="accpool", bufs=2) as accpool,
            tc.tile_pool(name="opool", bufs=2) as opool,
        ):
            for ct in range(NCT):
                rows = slice(128 * ct, 128 * (ct + 1))

                d = dpool.tile([128, COMBOS], F32)
                nc.sync.dma_start(d[:], u_ext[rows, :])
                cf = cfpool.tile([128, FF], F32)
                nc.sync.dma_start(cf[:], cf_ext[rows, :])
                nz = nzpool.tile([128, FF], F32)
                nc.sync.dma_start(nz[:], nz_ext[rows, :])

                # stage1+2 fused: h = min_j (a_j*d + b_j) as a chained
                # min-accumulate on vector (custom DVE op), ping-ponging
                # between the two halves of acc
                acc = accpool.tile([128, 2 * COMBOS], F32)
                nc.vector.tensor_scalar(
                    acc[:, 0:COMBOS], d[:], a[0], cf[:, 0:1],
                    op0=mybir.AluOpType.mult, op1=mybir.AluOpType.add,
                )
                for j in range(1, FF):
                    src = acc[:, (1 - j % 2) * COMBOS:][:, 0:COMBOS]
                    dst = acc[:, (j % 2) * COMBOS:][:, 0:COMBOS]
                    nc.vector._custom_dve(
                        fmin, out=dst, in0=d[:], in1=src,
                        s0=a[j], s1=cf[:, j:j + 1],
                    )
                h = acc[:, ((FF - 1) % 2) * COMBOS:][:, 0:COMBOS]

                # stage3: out[k] = relu(h - z_k)
                oa = opool.tile([128, FF * COMBOS], F32)
                for k in range(FF):
                    o = oa[:, k * COMBOS:(k + 1) * COMBOS]
                    if k < FF - S3_VEC:
                        nc.scalar.activation(
                            o, h[:], mybir.ActivationFunctionType.Relu,
                            bias=nz[:, k:k + 1], scale=1.0,
                        )
                    else:
                        nc.vector.tensor_scalar(
                            o, h[:], nz[:, k:k + 1], 0.0,
                            op0=mybir.AluOpType.add, op1=mybir.AluOpType.max,
                        )

                # stream output in 1MB chunks so stores overlap stage3 and
                # the final tile's store doesn't serialize after compute
                CH = 4 * COMBOS
                for c in range(4):
                    nc.sync.dma_start(
                        out_ext[rows, c * CH:(c + 1) * CH],
                        oa[:, c * CH:(c + 1) * CH],
                    )
    nc.finalize()
    return nc


def _prep_inputs(u_coarse, topo):
    """Host-side: per-cell sorted-prefix coefficients + per-core shards."""
    u = np.ascontiguousarray(np.asarray(u_coarse, dtype=np.float32))
    tp = np.asarray(topo, dtype=np.float32)
    # fold fine topo into per-coarse-cell patches [NY, NX, FF]
    z = tp.reshape(NY, F, NX, F).transpose(0, 2, 1, 3).reshape(NY, NX, FF)
    zs = np.sort(z.astype(np.float64), axis=-1)
    pref = np.cumsum(zs, axis=-1)
    jj = np.arange(1, FF + 1, dtype=np.float64)
    coef = (pref / jj).astype(np.float32)          # [NY, NX, FF]
    negz = (-z).astype(np.float32)                 # [NY, NX, FF]

    in_maps = []
    for c in range(NCORES):
        ys = slice(c * YPC, (c + 1) * YPC)
        u_core = np.ascontiguousarray(
            u[:, ys, :, :].transpose(1, 2, 0, 3)
        ).reshape(CELLS, COMBOS)
        cf_core = np.ascontiguousarray(coef[ys]).reshape(CELLS, FF)
        nz_core = np.ascontiguousarray(negz[ys]).reshape(CELLS, FF)
        in_maps.append({"u": u_core, "coef": cf_core, "negz": nz_core})
    return in_maps


def _unshard(results):
    out_all = np.stack([r["out"] for r in results])          # [8, 512, 8192]
    arr = out_all.reshape(NCORES, YPC, NX, F, F, BS, NT)      # c,yl,x,fy,fx,b,t
    arr = arr.transpose(5, 0, 1, 3, 2, 4, 6)                  # b,c,yl,fy,x,fx,t
    return np.ascontiguousarray(arr).reshape(BS, NY * F, NX * F, NT)


def kernel(u_coarse, topo):
    if "nc" not in _CACHE:
        _CACHE["nc"] = _build_nc()
    nc = _CACHE["nc"]
    in_maps = _prep_inputs(u_coarse, topo)
    res = run_bass_kernel_spmd(nc, in_maps, core_ids=list(range(NCORES)))
    return _unshard(res.results)


if __name__ == "__main__":
    import reference

    inputs = reference.setup_inputs()
    out = kernel(**{k: np.asarray(v) for k, v in inputs.items()})
    print("out", out.shape, out.dtype)


# revision 17
# speedup vs baseline: 1.1750x; 1.1750x over previous
"""Bathtub reconstructor Trainium2 kernel.

Reference does, per (b, y, x, t) cell with its 16 fine topo values z_k:
    solve mean(relu(h - z)) = d by 20-step bisection, output relu(h - z_k).

Key identity (water-filling): with z sorted ascending and P_j = z_1+...+z_j,
    sum_k relu(h - z_k) = max_j (j*h - P_j)   (convex, increasing)
so the root of sum = 16*d is exactly
    h* = min_{j=1..16} (16*d + P_j) / j = min_j (a_j * d + b_j),
with a_j = 16/j (constants) and b_j = P_j/j (per-cell constants).
This replaces the 20-iteration bisection with 16 fused multiply-adds and a
16-way min, then the output pass relu(h* - z_k).

Device layout (per core, n_y sharded 8 ways -> 8 y-rows/core):
  partitions = 128 cells (4 tiles cover the 512 (y,x) cells)
  free dim   = 512 combos (b-major: b*32 + t)
  stage1: hj[j] = a_j*d + b_j   (scalar ACT: Identity w/ scale+bias, and
                                 vector tensor_scalar mult+add, split)
  stage2: h = min over j        (vector tensor_reduce, j innermost via AP)
  stage3: out[k] = relu(h - z_k) (vector tensor_scalar add+max / ACT Relu)
All DMAs fully contiguous; host pre/post-permutes (cheap numpy).
"""

import numpy as np

import concourse.bass as bass
import concourse.tile as tile
from concourse import bacc, dve_ops, mybir
from concourse.bass_utils import run_bass_kernel_spmd
from concourse.dve_ops import OPS, DveOp, get_dve_sub_opcode, has_src1
from concourse.dve_spec import C0, C1, Spec, Src0, Src1, lower, minn
from concourse.dve_uop import DveOpSpec


def _register_affine_min() -> DveOp:
    """Custom fused DVE op: out = min(in0*s0 + s1, in1).

    One [128,512] 1x-rate instruction per water-level line replaces a
    tensor_scalar (affine) + tensor_tensor (min-tree level) pair: the
    16-line lower envelope becomes a 15-op min-accumulate chain.
    """
    for o in OPS:
        if o.name == "AFFINE_THEN_MIN":
            return o
    spec = Spec(
        body=minn(Src0 * C0 + C1, Src1),
        reference=lambda in0, in1, s0, s1, imm2: np.minimum(
            in0.astype(np.float32) * s0 + s1, in1
        ),
    )
    op = DveOp("AFFINE_THEN_MIN", spec, subdim=False, uops_sha={})
    OPS.append(op)
    dve_ops.CUSTOM_DVE_SPECS[op.name] = op.spec
    dve_ops._SUB_OPCODE_FOR_NAME[op.name] = (
        dve_ops._CUSTOM_DVE_ROW_BASE + len(OPS) - 1
    )
    for ver in ("v3", "v4"):
        tmp = DveOpSpec(
            name=op.name,
            opcode=get_dve_sub_opcode(op.name),
            uops=lower(spec, ver=ver),
            rd1_en=has_src1(spec),
        )
        op.uops_sha[ver] = tmp.sha(ver)
    return op

BS, NY, NX, NT, F = 16, 64, 64, 32, 4
FF = F * F                # 16 fine cells per coarse cell
NCORES = 8
YPC = NY // NCORES        # 8 coarse y rows per core
CELLS = YPC * NX          # 512 cells per core
NCT = CELLS // 128        # 4 cell-tiles of 128 partitions
COMBOS = BS * NT          # 512 (b, t) combos per cell

F32 = mybir.dt.float32

# Engine split: vector runs the fused affine+min chain (stage1+2), scalar
# runs stage3 relu acts (~707ns each). GpSimd is unusable here: its
# tensor_scalar path measured 8.3us/op and its SBUF-port contention
# starved the DVE 12x.
S3_VEC = 0    # stage3: last S3_VEC k's on vector, rest on scalar

_CACHE = {}


def _build_nc():
    fmin = _register_affine_min()
    nc = bacc.Bacc(
        "TRN2", target_bir_lowering=False, debug=False, num_devices=NCORES
    )
    u_ext = nc.declare_dram_parameter("u", [CELLS, COMBOS], F32, isOutput=False)
    cf_ext = nc.declare_dram_parameter("coef", [CELLS, FF], F32, isOutput=False)
    nz_ext = nc.declare_dram_parameter("negz", [CELLS, FF], F32, isOutput=False)
    out_ext = nc.declare_dram_parameter(
        "out", [CELLS, FF * COMBOS], F32, isOutput=True
    )

    a = [float(FF) / j for j in range(1, FF + 1)]

    with tile.TileContext(nc) as tc:
        with (
            tc.tile_pool(name="dpool", bufs=4) as dpool,
            tc.tile_pool(name="cfpool", bufs=4) as cfpool,
            tc.tile_pool(name="nzpool", bufs=4) as nzpool,
            tc.tile_pool(name="accpool", bufs=4) as accpool,
            tc.tile_pool(name="opool", bufs=3) as opool,
        ):
            for ct in range(NCT):
                rows = slice(128 * ct, 128 * (ct + 1))

                d = dpool.tile([128, COMBOS], F32)
                nc.sync.dma_start(d[:], u_ext[rows, :])
                cf = cfpool.tile([128, FF], F32)
                nc.sync.dma_start(cf[:], cf_ext[rows, :])
                nz = nzpool.tile([128, FF], F32)
                nc.sync.dma_start(nz[:], nz_ext[rows, :])

                # stage1+2 fused: h = min_j (a_j*d + b_j) as a chained
                # min-accumulate on vector (custom DVE op), ping-ponging
                # between the two halves of acc
                acc = accpool.tile([128, 2 * COMBOS], F32)
                nc.vector.tensor_scalar(
                    acc[:, 0:COMBOS], d[:], a[0], cf[:, 0:1],
                    op0=mybir.AluOpType.mult, op1=mybir.AluOpType.add,
                )
                for j in range(1, FF):
                    src = acc[:, (1 - j % 2) * COMBOS:][:, 0:COMBOS]
                    dst = acc[:, (j % 2) * COMBOS:][:, 0:COMBOS]
                    nc.vector._custom_dve(
                        fmin, out=dst, in0=d[:], in1=src,
                        s0=a[j], s1=cf[:, j:j + 1],
                    )
                h = acc[:, ((FF - 1) % 2) * COMBOS:][:, 0:COMBOS]

                # stage3: out[k] = relu(h - z_k)
                oa = opool.tile([128, FF * COMBOS], F32)
                for k in range(FF):
                    o = oa[:, k * COMBOS:(k + 1) * COMBOS]
                    if k < FF - S3_VEC:
                        nc.scalar.activation(
                            o, h[:], mybir.ActivationFunctionType.Relu,
                            bias=nz[:, k:k + 1], scale=1.0,
                        )
                    else:
                        nc.vector.tensor_scalar(
                            o, h[:], nz[:, k:k + 1], 0.0,
                            op0=mybir.AluOpType.add, op1=mybir.AluOpType.max,
                        )

                # stream output in 1MB chunks so stores overlap stage3 and
                # the final tile's store doesn't serialize after compute
                CH = 4 * COMBOS
                for c in range(4):
                    nc.sync.dma_start(
                        out_ext[rows, c * CH:(c + 1) * CH],
                        oa[:, c * CH:(c + 1) * CH],
                    )
    nc.finalize()
    return nc


def _prep_inputs(u_coarse, topo):
    """Host-side: per-cell sorted-prefix coefficients + per-core shards."""
    u = np.ascontiguousarray(np.asarray(u_coarse, dtype=np.float32))
    tp = np.asarray(topo, dtype=np.float32)
    # fold fine topo into per-coarse-cell patches [NY, NX, FF]
    z = tp.reshape(NY, F, NX, F).transpose(0, 2, 1, 3).reshape(NY, NX, FF)
    zs = np.sort(z.astype(np.float64), axis=-1)
    pref = np.cumsum(zs, axis=-1)
    jj = np.arange(1, FF + 1, dtype=np.float64)
    coef = (pref / jj).astype(np.float32)          # [NY, NX, FF]
    negz = (-z).astype(np.float32)                 # [NY, NX, FF]

    in_maps = []
    for c in range(NCORES):
        ys = slice(c * YPC, (c + 1) * YPC)
        u_core = np.ascontiguousarray(
            u[:, ys, :, :].transpose(1, 2, 0, 3)
        ).reshape(CELLS, COMBOS)
        cf_core = np.ascontiguousarray(coef[ys]).reshape(CELLS, FF)
        nz_core = np.ascontiguousarray(negz[ys]).reshape(CELLS, FF)
        in_maps.append({"u": u_core, "coef": cf_core, "negz": nz_core})
    return in_maps


def _unshard(results):
    out_all = np.stack([r["out"] for r in results])          # [8, 512, 8192]
    arr = out_all.reshape(NCORES, YPC, NX, F, F, BS, NT)      # c,yl,x,fy,fx,b,t
    arr = arr.transpose(5, 0, 1, 3, 2, 4, 6)                  # b,c,yl,fy,x,fx,t
    return np.ascontiguousarray(arr).reshape(BS, NY * F, NX * F, NT)


def kernel(u_coarse, topo):
    if "nc" not in _CACHE:
        _CACHE["nc"] = _build_nc()
    nc = _CACHE["nc"]
    in_maps = _prep_inputs(u_coarse, topo)
    res = run_bass_kernel_spmd(nc, in_maps, core_ids=list(range(NCORES)))
    return _unshard(res.results)


if __name__ == "__main__":
    import reference

    inputs = reference.setup_inputs()
    out = kernel(**{k: np.asarray(v) for k, v in inputs.items()})
    print("out", out.shape, out.dtype)
